# revision 19
# baseline (speedup 1.0000x reference)
"""Multi-head attention (B=4, S=2048, D=1024, H=16, causal) on 8 TRN2 NeuronCores.

Sharding: batch x head-group (Megatron).  Core c handles batch c//2 and head
group c%2 (8 heads = 512 of the 1024 hidden dims).  w_q/w_k/w_v are
column-parallel, w_o row-parallel; the two partial outputs per batch are summed
on the host during unsharding.

Device kernel (per core, all matmuls bf16, fp32 accumulation), structured as a
software pipeline that keeps both the PE array and the activation engine busy
end to end:
  - attention iterates head-pair-outer / query-chunk-inner, so the ScalarE exp
    stream starts as soon as head-pair 0 is projected and stays steady for the
    whole kernel instead of piling up at the end
  - all remaining projection work (q/k/v head-pairs 1-3, o-proj) is a pool of
    fill tasks popped between attention j-steps with a credit pacer, plugging
    the PE idle slots that the exp latency would otherwise leave
  - scoresT[k,q] = kT.T @ qT per head, two heads row-packed concurrently on the
    PE array (64-partition lhsT at base 0/64 -> tile_position row groups)
  - attn@V uses a [V | ones] stationary access pattern (step-sliced into a
    9-slot tile, slot 8 = ones shared by all k-tiles) so the softmax
    denominator accumulates in PSUM partitions 64:128 of the same matmul
  - softmax normalization via DVE reciprocal_approx_fast (~51 ULP) + tensor_mul
    instead of ScalarE Ln/Exp: keeps the activation engine exp-only
  - projection evacuations ride ScalarE (Copy / Identity+bias) early when it is
    idle and VectorE later, balancing both engines under the PE roofline
  - DMA issue order is consumption order: wq then xq k-tiles (wavefront
    consumes them as they land), xk halves, xv in 512-token chunks (ring of 2)
  - b_q added on qT evacuation, b_k dropped (cancels in softmax), b_v folded
    into b_o on host
"""

import os
import sys

for _p in ("/opt/trn_rl_repo",):
    if _p not in sys.path and os.path.isdir(_p):
        sys.path.insert(0, _p)

from contextlib import ExitStack

import ml_dtypes
import numpy as np

import concourse.bass as bass
import concourse.tile as tile
from concourse import bacc, mybir
from concourse import bass_utils

BF16 = ml_dtypes.bfloat16

B = 4
S = 2048
D = 1024
H = 16
DK = 64
NCORES = 8
DL = D // 2  # local (per head-group) hidden dims = 512
NHP = 4  # head pairs per core
KT = D // 128  # contraction tiles over model dim = 8
TT = S // 128  # token tiles = 16
QC = S // 512  # query chunks of 512 = 4

FP32 = mybir.dt.float32
DTBF = mybir.dt.bfloat16
_DEBUG_DUMP = False


class _Fills:
    """Ordered pool of PE fill tasks popped between attention j-steps.

    Each item is (key, cost_ns, fn).  step(credit_ns) accumulates credit and
    emits items while affordable; run_until(key) force-emits through key
    (dependency barrier before an attention block that needs that tile)."""

    def __init__(self):
        self.items = []
        self.idx = 0
        self.credit = 0.0
        self.done = set()

    def add(self, key, cost, fn):
        self.items.append((key, cost, fn))

    def _emit_next(self):
        key, cost, fn = self.items[self.idx]
        self.idx += 1
        fn()
        self.done.add(key)
        return cost

    def step(self, credit):
        self.credit += credit
        while self.idx < len(self.items):
            cost = self.items[self.idx][1]
            if self.credit < cost:
                break
            self.credit -= self._emit_next()

    def run_until(self, *keys):
        need = [k for k in keys if k not in self.done]
        while need:
            self._emit_next()
            self.credit = 0.0
            need = [k for k in need if k not in self.done]

    def drain(self):
        while self.idx < len(self.items):
            self._emit_next()
        self.credit = 0.0


def _emit(nc, causal: bool):
    # x tensors are host-prepacked chunk-major so each chunk is one
    # fully-contiguous DMA: xq/xv as [QC, 128, KT, 512], xk as [2, 128, KT, 1024]
    xq = nc.dram_tensor("xq_t", [QC, 128, KT, 512], DTBF, kind="ExternalInput").ap()
    xk = nc.dram_tensor("xk_t", [QC, 128, KT, 512], DTBF, kind="ExternalInput").ap()
    xv = nc.dram_tensor("xv_t", [QC, 128, KT, 512], DTBF, kind="ExternalInput").ap()
    wq_t = nc.dram_tensor("wq_p", [128, KT, DL], DTBF, kind="ExternalInput").ap()
    wk_t = nc.dram_tensor("wk_p", [128, KT, DL], DTBF, kind="ExternalInput").ap()
    wv_t = nc.dram_tensor("wv_p", [128, KT, DL], DTBF, kind="ExternalInput").ap()
    wo_t = nc.dram_tensor("wo_p", [128, NHP, D], DTBF, kind="ExternalInput").ap()
    bq_t = nc.dram_tensor("bq_t", [128, 4], FP32, kind="ExternalInput").ap()
    bo_t = nc.dram_tensor("bo_t", [128, 8], FP32, kind="ExternalInput").ap()
    out_pt = nc.dram_tensor("out_pt", [D, S], DTBF, kind="ExternalOutput").ap()

    EXP = mybir.ActivationFunctionType.Exp
    IDENT = mybir.ActivationFunctionType.Identity

    with tile.TileContext(nc) as tc, ExitStack() as ctx:
        consts = ctx.enter_context(tc.tile_pool(name="consts", bufs=1))
        xq_pool = ctx.enter_context(tc.tile_pool(name="xq", bufs=1))
        xk_pool = ctx.enter_context(tc.tile_pool(name="xk", bufs=1))
        xv_pool = ctx.enter_context(tc.tile_pool(name="xv", bufs=2))
        qkv_pool = ctx.enter_context(tc.tile_pool(name="qkv", bufs=1))
        et_pool = ctx.enter_context(tc.tile_pool(name="et", bufs=3))
        rc_pool = ctx.enter_context(tc.tile_pool(name="rc", bufs=2))
        out_pool = ctx.enter_context(tc.tile_pool(name="osb", bufs=6))
        ps_s = ctx.enter_context(tc.tile_pool(name="ps_s", bufs=2, space="PSUM"))
        ps_acc = ctx.enter_context(tc.tile_pool(name="ps_acc", bufs=1, space="PSUM"))
        ps_op = ctx.enter_context(tc.tile_pool(name="ps_op", bufs=2, space="PSUM"))

        # ---- weight / bias SBUF tiles ----
        wq_sb = consts.tile([128, KT, DL], DTBF)
        wk_sb = consts.tile([128, KT, DL], DTBF)
        wv_sb = consts.tile([128, KT, DL], DTBF)
        wo_sb = consts.tile([128, NHP, D], DTBF)
        bq_sb = consts.tile([128, 4], FP32)
        bo_sb = consts.tile([128, 8], FP32)

        # chunk-major free dims: DMA destinations are fully contiguous
        xqt = xq_pool.tile([128, QC, KT, 512], DTBF)
        xkt = xk_pool.tile([128, QC, KT, 512], DTBF)

        # qT/kT are 2-slot rings over head pairs: pass hp reads slot hp%2 while
        # the fills for pass hp+1 write slot (hp+1)%2
        qT_sb = qkv_pool.tile([128, 2, S], DTBF)
        kT_sb = qkv_pool.tile([128, 2, S], DTBF)
        # [ones | V] per head: vp_sb[:, tt, h, 0:64] = 1.0, [.., 64:128] = V.
        # ones first so the softmax denominator lands on PSUM partitions 0:64 —
        # the custom-DVE reciprocal misreads PSUM at partition base 64 on HW.
        vp_sb = qkv_pool.tile([128, TT, 8, 128], DTBF)
        a_sb = qkv_pool.tile([128, NHP, S], DTBF)

        nc.vector.memset(vp_sb[:, :, :, 0:64], 1.0)

        # lower-triangle-inclusive (k <= q) binary mask for diagonal tiles
        tri_sb = consts.tile([128, 2, 128], DTBF)
        nc.gpsimd.memset(tri_sb[:], 1.0)
        for h2 in range(2):
            nc.gpsimd.affine_select(
                out=tri_sb[:, h2, :],
                in_=tri_sb[:, h2, :],
                compare_op=mybir.AluOpType.is_ge,
                fill=0.0,
                base=0,
                pattern=[[1, 128]],
                channel_multiplier=-1,
            )

        # ---- DMA issue order == consumption order --------------------------
        # one contiguous transfer per chunk, spread over the three DGE queues
        # (sync / scalar / gpsimd) so the prelude inputs land in ~15us.
        # xv chunks 2/3 are allocated lazily (after chunk 0/1 readers exist)
        # to keep the ring-reuse ordering acyclic.
        xv_chunks = [
            xv_pool.tile([128, KT, 512], DTBF, tag="xv", name=f"xv{i}") for i in range(2)
        ]
        # ~90GB/s per DGE queue: deal 0.5MB half-chunks round-robin over the
        # three queues in strict consumption-priority order
        def _half(dst, src_ap, n=8):
            h = n // 2
            return [
                (lambda e, d=dst, s=src_ap: e.dma_start(d[:, 0:h], s[:, 0:h])),
                (lambda e, d=dst, s=src_ap: e.dma_start(d[:, h:n], s[:, h:n])),
            ]

        items = []
        items += _half(wq_sb, wq_t)
        items += _half(xqt[:, 0], xq[0])
        items += _half(wk_sb, wk_t)
        items += _half(xkt[:, 0], xk[0])
        items += _half(wv_sb, wv_t)
        items += _half(xv_chunks[0], xv[0])
        items += _half(xqt[:, 1], xq[1])
        items += _half(xkt[:, 1], xk[1])
        items += _half(xv_chunks[1], xv[1])
        items.append(lambda e: e.dma_start(bq_sb[:], bq_t[:]))
        items += _half(xqt[:, 2], xq[2])
        items += _half(xkt[:, 2], xk[2])
        items += _half(xqt[:, 3], xq[3])
        items += _half(xkt[:, 3], xk[3])
        items += _half(wo_sb, wo_t, n=4)
        items.append(lambda e: e.dma_start(bo_sb[:], bo_t[:]))
        engines = (nc.gpsimd, nc.sync, nc.scalar)
        for i, it in enumerate(items):
            it(engines[i % 3])

        # ---- projection unit emitters --------------------------------------
        def qp_unit(ot, tcid, ev_scalar):
            ps = ps_op.tile([128, 512], FP32, tag="op", name="ps")
            for k in range(KT):
                nc.tensor.matmul(
                    ps[:],
                    wq_sb[:, k, ot * 128 : (ot + 1) * 128],
                    xqt[:, tcid, k, :],
                    start=(k == 0),
                    stop=(k == KT - 1),
                )
            dst = qT_sb[:, ot % 2, tcid * 512 : (tcid + 1) * 512]
            if ev_scalar:
                nc.scalar.activation(dst, ps[:], IDENT, bias=bq_sb[:, ot : ot + 1], scale=1.0)
            else:
                nc.vector.tensor_scalar_add(dst, ps[:], bq_sb[:, ot : ot + 1])

        def kp_unit(ot, tcid, ev_scalar):
            ps = ps_op.tile([128, 512], FP32, tag="op", name="ps")
            for k in range(KT):
                nc.tensor.matmul(
                    ps[:],
                    wk_sb[:, k, ot * 128 : (ot + 1) * 128],
                    xkt[:, tcid, k, :],
                    start=(k == 0),
                    stop=(k == KT - 1),
                )
            dst = kT_sb[:, ot % 2, tcid * 512 : (tcid + 1) * 512]
            if ev_scalar:
                nc.scalar.copy(dst, ps[:])
            else:
                nc.vector.tensor_copy(dst, ps[:])

        def vp_unit(tt, ev_scalar):
            xvc = xv_chunks[tt // 4]
            ps = ps_op.tile([128, 512], FP32, tag="op", name="ps")
            for k in range(KT):
                nc.tensor.matmul(
                    ps[:],
                    xvc[:, k, (tt % 4) * 128 : (tt % 4 + 1) * 128],
                    wv_sb[:, k, :],
                    start=(k == 0),
                    stop=(k == KT - 1),
                )
            dst = vp_sb[:, tt, :, 64:128]
            if ev_scalar:
                nc.scalar.copy(dst, ps[:])
            else:
                nc.vector.tensor_copy(dst, ps[:])

        def op_unit(qc, od):
            ps = ps_op.tile([128, 512], FP32, tag="op", name="ps")
            for hp in range(NHP):
                nc.tensor.matmul(
                    ps[:],
                    wo_sb[:, hp, od * 128 : (od + 1) * 128],
                    a_sb[:, hp, qc * 512 : (qc + 1) * 512],
                    start=(hp == 0),
                    stop=(hp == NHP - 1),
                )
            osb = out_pool.tile([128, 512], DTBF, tag="osb", name="osb")
            nc.vector.tensor_scalar_add(osb[:], ps[:], bo_sb[:, od : od + 1])
            eng = nc.sync if od % 2 == 0 else nc.gpsimd
            eng.dma_start(
                out_pt[od * 128 : (od + 1) * 128, qc * 512 : (qc + 1) * 512], osb[:]
            )

        # ---- PE warm-up: a short junk-matmul burst while input DMAs stream,
        # so HAM is at 8/8 when the real pipeline starts ----------------------
        warm = consts.tile([128, 512], DTBF)
        nc.vector.memset(warm[:], 0.0)
        wps = ps_s.tile([128, 2, 512], FP32, tag="ps_s", name="warmps")
        for i in range(16):
            nc.tensor.matmul(wps[:, 0, :], warm[:, 0:128], warm[:], start=True, stop=True)

        # ---- prelude: just enough projection for attn(0,0) ------------------
        qp_unit(0, 0, ev_scalar=True)
        kp_unit(0, 0, ev_scalar=True)
        for tt in range(4):
            vp_unit(tt, ev_scalar=True)
        xv_chunks.append(xv_pool.tile([128, KT, 512], DTBF, tag="xv", name="xv2"))
        nc.scalar.dma_start(xv_chunks[2][:], xv[2])

        # ---- fill pool ------------------------------------------------------
        fills = _Fills()
        QP_C, KP_C, VP_C, OP_C = 2200.0, 2200.0, 2200.0, 1550.0

        def add_pass0():
            f = fills
            f.add("kp0_1", KP_C, lambda: kp_unit(0, 1, False))
            f.add("qp0_1", QP_C, lambda: qp_unit(0, 1, False))
            f.add("vp4", VP_C, lambda: vp_unit(4, False))
            f.add("vp5", VP_C, lambda: vp_unit(5, False))
            f.add("vp6", VP_C, lambda: vp_unit(6, False))
            f.add("vp7", VP_C, lambda: vp_unit(7, False))

            def _xvc3():
                xv_chunks.append(
                    xv_pool.tile([128, KT, 512], DTBF, tag="xv", name="xv3")
                )
                nc.gpsimd.dma_start(xv_chunks[3][:], xv[3])

            f.add("xvc3", 0.0, _xvc3)
            f.add("qp1_0", QP_C, lambda: qp_unit(1, 0, False))
            f.add("kp1_0", KP_C, lambda: kp_unit(1, 0, False))
            f.add("qp0_2", QP_C, lambda: qp_unit(0, 2, False))
            f.add("kp0_2", KP_C, lambda: kp_unit(0, 2, False))
            f.add("vp8", VP_C, lambda: vp_unit(8, False))
            f.add("vp9", VP_C, lambda: vp_unit(9, False))
            f.add("vp10", VP_C, lambda: vp_unit(10, False))
            f.add("vp11", VP_C, lambda: vp_unit(11, False))
            f.add("qp1_1", QP_C, lambda: qp_unit(1, 1, False))
            f.add("kp1_1", KP_C, lambda: kp_unit(1, 1, False))
            f.add("qp0_3", QP_C, lambda: qp_unit(0, 3, False))
            f.add("kp0_3", KP_C, lambda: kp_unit(0, 3, False))
            for tt in range(12, 16):
                f.add(f"vp{tt}", VP_C, lambda tt=tt: vp_unit(tt, False))
            for tcid in range(2, 4):
                f.add(f"qp1_{tcid}", QP_C, lambda t=tcid: qp_unit(1, t, False))
                f.add(f"kp1_{tcid}", KP_C, lambda t=tcid: kp_unit(1, t, False))

        def add_passN(hp_next):
            # emitted as pass (hp_next-1) fills; evac on vector
            for tcid in range(4):
                fills.add(
                    f"qp{hp_next}_{tcid}", QP_C, lambda o=hp_next, t=tcid: qp_unit(o, t, False)
                )
                fills.add(
                    f"kp{hp_next}_{tcid}", KP_C, lambda o=hp_next, t=tcid: kp_unit(o, t, False)
                )

        add_pass0()

        # ---- attention block for one (query chunk, head pair) ---------------
        def attn(qc, hp, credit_scale=1.0):
            jmax = 4 * qc + 3 if causal else TT - 1
            q0 = qc * 512
            pso = ps_acc.tile([128, 2, 512], FP32, tag="acc", name="pso")

            def offof(j):
                r = j - 4 * qc if causal else -1
                return 128 * r if r >= 0 else 0

            def scores(j):
                off = offof(j)
                pss = ps_s.tile([128, 2, 512], FP32, tag="ps_s", name="pss")
                for h2 in range(2):
                    nc.tensor.matmul(
                        pss[:, h2, off:512],
                        kT_sb[h2 * 64 : (h2 + 1) * 64, hp % 2, j * 128 : (j + 1) * 128],
                        qT_sb[h2 * 64 : (h2 + 1) * 64, hp % 2, q0 + off : q0 + 512],
                        start=True,
                        stop=True,
                    )
                et = et_pool.tile([128, 2, 512], DTBF, tag="et", name="et")
                nc.scalar.activation(et[:, :, off:], pss[:, :, off:], EXP, scale=0.125)
                if off or (causal and j == 4 * qc):
                    # zero where k (partition) > q (free col), both heads
                    nc.vector.tensor_mul(
                        et[:, :, off : off + 128], et[:, :, off : off + 128], tri_sb[:]
                    )
                return et

            et_next = scores(0)
            for j in range(jmax + 1):
                off = offof(j)
                et = et_next
                if j < jmax:
                    et_next = scores(j + 1)
                for h2 in range(2):
                    # rows 0:64 accumulate the softmax denominator (ones
                    # block), rows 64:128 attn@V.  Causally-trimmed widths on
                    # interleaved chains; per-element has_written semantics
                    # make this safe but the sim's zero-region tracker
                    # can't express it.
                    nc.tensor.matmul(
                        pso[:, h2, off:512],
                        vp_sb[:, j, 2 * hp + h2, :],
                        et[:, h2, off:],
                        start=(j == 0),
                        stop=(j == jmax),
                        skip_group_check=True,
                    )
                w = 512 - off
                fills.step((0.42 * w + 47.0) * credit_scale)
            # normalize: 1/den on DVE (recip ~51 ULP), then scale the V rows
            rc = rc_pool.tile([128, 2, 512], FP32, tag="rc", name="rc")
            nc.vector.reciprocal_approx_fast(rc[0:64, :, :], pso[0:64, :, :])
            for h2 in range(2):
                nc.vector.tensor_mul(
                    a_sb[h2 * 64 : (h2 + 1) * 64, hp, qc * 512 : (qc + 1) * 512],
                    pso[64:128, h2, :],
                    rc[0:64, h2, :],
                )

        # ---- main pipeline: head-pair passes --------------------------------
        def barrier(qc, hp):
            keys = []
            if causal:
                tt_hi = 4 * qc + 3
                tc_hi = qc
            else:
                tt_hi = TT - 1
                tc_hi = 3
            if hp == 0:
                keys += [f"vp{t}" for t in range(4, tt_hi + 1)]
                keys += [f"kp0_{t}" for t in range(1, tc_hi + 1)]
                if qc >= 1:
                    keys += [f"qp0_{qc}"]
            else:
                keys += [f"qp{hp}_{qc}"]
                keys += [f"kp{hp}_{t}" for t in range(0, tc_hi + 1)]
            return keys

        for hp in range(NHP):
            if 1 <= hp < NHP - 1:
                # pass hp+1's q/k projections (pass 1's are already in the
                # pass-0 list via add_pass0)
                add_passN(hp + 1)
            for qc in range(QC):
                fills.run_until(*barrier(qc, hp))
                attn(qc, hp, credit_scale=(3.0 if hp == NHP - 1 else 1.0))
                if hp == NHP - 1 and qc < QC - 1:
                    # finished chunk's o-proj becomes pass-3 fill work
                    for od in range(8):
                        fills.add(f"op{qc}_{od}", OP_C, lambda q=qc, o=od: op_unit(q, o))

        fills.drain()

        if _DEBUG_DUMP:
            dbg_a = nc.dram_tensor("dbg_a", [128, NHP, S], DTBF, kind="ExternalOutput").ap()
            dbg_q = nc.dram_tensor("dbg_q", [128, 2, S], DTBF, kind="ExternalOutput").ap()
            dbg_k = nc.dram_tensor("dbg_k", [128, 2, S], DTBF, kind="ExternalOutput").ap()
            dbg_v = nc.dram_tensor("dbg_v", [128, TT, 8, 128], DTBF, kind="ExternalOutput").ap()
            nc.sync.dma_start(dbg_a[:], a_sb[:])
            nc.sync.dma_start(dbg_q[:], qT_sb[:])
            nc.sync.dma_start(dbg_k[:], kT_sb[:])
            nc.sync.dma_start(dbg_v[:], vp_sb[:])

        # ---- tail: last chunk's o-proj with all 8 PSUM banks as chains ------
        fin = [ps_s.tile([128, 2, 512], FP32, tag="ps_s", name=f"fin{i}") for i in range(2)]
        fin_acc = ps_acc.tile([128, 2, 512], FP32, tag="acc", name="fin_acc")
        fin_op = [ps_op.tile([128, 512], FP32, tag="op", name=f"finop{i}") for i in range(2)]
        qc = QC - 1
        chains = [
            fin[0][:, 0, :], fin[0][:, 1, :], fin[1][:, 0, :], fin[1][:, 1, :],
            fin_acc[:, 0, :], fin_acc[:, 1, :], fin_op[0][:], fin_op[1][:],
        ]
        for hp in range(NHP):
            for od in range(8):
                nc.tensor.matmul(
                    chains[od],
                    wo_sb[:, hp, od * 128 : (od + 1) * 128],
                    a_sb[:, hp, qc * 512 : (qc + 1) * 512],
                    start=(hp == 0),
                    stop=(hp == NHP - 1),
                )
        for od in range(8):
            osb = out_pool.tile([128, 512], DTBF, tag="osb", name="osb")
            if od % 2 == 0:
                nc.vector.tensor_scalar_add(osb[:], chains[od], bo_sb[:, od : od + 1])
            else:
                nc.scalar.activation(osb[:], chains[od], IDENT, bias=bo_sb[:, od : od + 1], scale=1.0)
            eng = (nc.sync, nc.gpsimd, nc.scalar)[od % 3]
            eng.dma_start(
                out_pt[od * 128 : (od + 1) * 128, qc * 512 : (qc + 1) * 512], osb[:]
            )


_CACHE = {}


def _get_compiled(causal: bool):
    key = bool(causal)
    if key not in _CACHE:
        nc = bacc.Bacc("TRN2", target_bir_lowering=False, debug=False, num_devices=NCORES)
        _emit(nc, causal=key)
        nc.compile()
        _CACHE[key] = nc
    return _CACHE[key]


def make_in_maps(query, key, value, w_q, b_q, w_k, b_k, w_v, b_v, w_o, b_o):
    """Build the per-core input maps (host-side sharding + layout prep)."""
    in_maps = []
    # b_v folds into the output bias: softmax rows sum to 1, so
    # attn(V + b_v) = attn(V) + b_v, and (A + b_v) @ w_o.T = A @ w_o.T + w_o @ b_v.
    # b_k drops entirely: scores shift constant along k cancels in softmax.
    bo_eff = (b_o + w_o.astype(np.float64) @ b_v.astype(np.float64)).astype(np.float32)
    for c in range(NCORES):
        b, hg = divmod(c, 2)
        sl = slice(hg * DL, (hg + 1) * DL)
        bo_core = bo_eff if hg == 0 else np.zeros_like(bo_eff)
        in_maps.append(
            {
                "xq_t": np.ascontiguousarray(
                    query[b].T.reshape(KT, 128, QC, 512).transpose(2, 1, 0, 3)).astype(BF16),
                "xk_t": np.ascontiguousarray(
                    key[b].T.reshape(KT, 128, QC, 512).transpose(2, 1, 0, 3)).astype(BF16),
                "xv_t": np.ascontiguousarray(
                    value[b].T.reshape(KT, 128, QC, 512).transpose(2, 1, 0, 3)).astype(BF16),
                "wq_p": np.ascontiguousarray(
                    w_q[sl, :].T.reshape(KT, 128, DL).transpose(1, 0, 2)).astype(BF16),
                "wk_p": np.ascontiguousarray(
                    w_k[sl, :].T.reshape(KT, 128, DL).transpose(1, 0, 2)).astype(BF16),
                "wv_p": np.ascontiguousarray(
                    w_v[sl, :].T.reshape(KT, 128, DL).transpose(1, 0, 2)).astype(BF16),
                "wo_p": np.ascontiguousarray(
                    w_o[:, sl].T.reshape(NHP, 128, D).transpose(1, 0, 2)).astype(BF16),
                "bq_t": np.ascontiguousarray(b_q[sl].reshape(4, 128).T).astype(np.float32),
                "bo_t": np.ascontiguousarray(bo_core.reshape(8, 128).T).astype(np.float32),
            }
        )
    return in_maps


def _mask_is_causal(mask):
    m = np.asarray(mask).reshape(S, S)
    return bool(np.array_equal(m, np.triu(np.ones((S, S), bool), k=1)))


def _mask_is_empty(mask):
    return not np.asarray(mask).any()


def kernel(query, key, value, mask, w_q, b_q, w_k, b_k, w_v, b_v, w_o, b_o, **_unused):
    query = np.asarray(query, np.float32)
    key = np.asarray(key, np.float32)
    value = np.asarray(value, np.float32)
    if _mask_is_causal(mask):
        causal = True
    elif _mask_is_empty(mask):
        causal = False
    else:
        raise NotImplementedError("only causal or empty masks are supported")

    nc = _get_compiled(causal)
    in_maps = make_in_maps(
        query, key, value,
        np.asarray(w_q, np.float32), np.asarray(b_q, np.float32),
        np.asarray(w_k, np.float32), np.asarray(b_k, np.float32),
        np.asarray(w_v, np.float32), np.asarray(b_v, np.float32),
        np.asarray(w_o, np.float32), np.asarray(b_o, np.float32),
    )
    res = bass_utils.run_bass_kernel_spmd(nc, in_maps, core_ids=list(range(NCORES)))
    out = np.empty((B, S, D), np.float32)
    for b in range(B):
        acc = (
            res.results[2 * b]["out_pt"].astype(np.float32)
            + res.results[2 * b + 1]["out_pt"].astype(np.float32)
        )
        out[b] = acc.T
    return out


# revision 20
# speedup vs baseline: 1.0136x; 1.0136x over previous
"""Multi-head attention (B=4, S=2048, D=1024, H=16, causal) on 8 TRN2 NeuronCores.

Sharding: batch x head-group (Megatron).  Core c handles batch c//2 and head
group c%2 (8 heads = 512 of the 1024 hidden dims).  w_q/w_k/w_v are
column-parallel, w_o row-parallel; the two partial outputs per batch are summed
on the host during unsharding.

Device kernel (per core, all matmuls bf16, fp32 accumulation), structured as a
software pipeline that keeps both the PE array and the activation engine busy
end to end:
  - attention iterates head-pair-outer / query-chunk-inner, so the ScalarE exp
    stream starts as soon as head-pair 0 is projected and stays steady for the
    whole kernel instead of piling up at the end
  - all remaining projection work (q/k/v head-pairs 1-3, o-proj) is a pool of
    fill tasks popped between attention j-steps with a credit pacer, plugging
    the PE idle slots that the exp latency would otherwise leave
  - scoresT[k,q] = kT.T @ qT per head, two heads row-packed concurrently on the
    PE array (64-partition lhsT at base 0/64 -> tile_position row groups)
  - attn@V uses a [V | ones] stationary access pattern (step-sliced into a
    9-slot tile, slot 8 = ones shared by all k-tiles) so the softmax
    denominator accumulates in PSUM partitions 64:128 of the same matmul
  - softmax normalization via DVE reciprocal_approx_fast (~51 ULP) + tensor_mul
    instead of ScalarE Ln/Exp: keeps the activation engine exp-only
  - projection evacuations ride ScalarE (Copy / Identity+bias) early when it is
    idle and VectorE later, balancing both engines under the PE roofline
  - DMA issue order is consumption order: wq then xq k-tiles (wavefront
    consumes them as they land), xk halves, xv in 512-token chunks (ring of 2)
  - b_q added on qT evacuation, b_k dropped (cancels in softmax), b_v folded
    into b_o on host
"""

import os
import sys

for _p in ("/opt/trn_rl_repo",):
    if _p not in sys.path and os.path.isdir(_p):
        sys.path.insert(0, _p)

from contextlib import ExitStack

import ml_dtypes
import numpy as np

import concourse.bass as bass
import concourse.tile as tile
from concourse import bacc, mybir
from concourse import bass_utils

BF16 = ml_dtypes.bfloat16

B = 4
S = 2048
D = 1024
H = 16
DK = 64
NCORES = 8
DL = D // 2  # local (per head-group) hidden dims = 512
NHP = 4  # head pairs per core
KT = D // 128  # contraction tiles over model dim = 8
TT = S // 128  # token tiles = 16
QC = S // 512  # query chunks of 512 = 4

FP32 = mybir.dt.float32
DTBF = mybir.dt.bfloat16
_DEBUG_DUMP = False


class _Fills:
    """Ordered pool of PE fill tasks popped between attention j-steps.

    Each item is (key, cost_ns, fn).  step(credit_ns) accumulates credit and
    emits items while affordable; run_until(key) force-emits through key
    (dependency barrier before an attention block that needs that tile)."""

    def __init__(self):
        self.items = []
        self.idx = 0
        self.credit = 0.0
        self.done = set()

    def add(self, key, cost, fn):
        self.items.append((key, cost, fn))

    def _emit_next(self):
        key, cost, fn = self.items[self.idx]
        self.idx += 1
        fn()
        self.done.add(key)
        return cost

    def step(self, credit):
        self.credit += credit
        while self.idx < len(self.items):
            cost = self.items[self.idx][1]
            if self.credit < cost:
                break
            self.credit -= self._emit_next()

    def run_until(self, *keys):
        need = [k for k in keys if k not in self.done]
        while need:
            self._emit_next()
            self.credit = 0.0
            need = [k for k in need if k not in self.done]

    def drain(self):
        while self.idx < len(self.items):
            self._emit_next()
        self.credit = 0.0


def _emit(nc, causal: bool):
    # x tensors are host-prepacked chunk-major so each chunk is one
    # fully-contiguous DMA: xq/xv as [QC, 128, KT, 512], xk as [2, 128, KT, 1024]
    xq = nc.dram_tensor("xq_t", [QC, 128, KT, 512], DTBF, kind="ExternalInput").ap()
    xk = nc.dram_tensor("xk_t", [QC, 128, KT, 512], DTBF, kind="ExternalInput").ap()
    xv = nc.dram_tensor("xv_t", [QC, 128, KT, 512], DTBF, kind="ExternalInput").ap()
    wq_t = nc.dram_tensor("wq_p", [128, KT, DL], DTBF, kind="ExternalInput").ap()
    wk_t = nc.dram_tensor("wk_p", [128, KT, DL], DTBF, kind="ExternalInput").ap()
    wv_t = nc.dram_tensor("wv_p", [128, KT, DL], DTBF, kind="ExternalInput").ap()
    wo_t = nc.dram_tensor("wo_p", [128, NHP, D], DTBF, kind="ExternalInput").ap()
    bq_t = nc.dram_tensor("bq_t", [128, 4], FP32, kind="ExternalInput").ap()
    bo_t = nc.dram_tensor("bo_t", [128, 8], FP32, kind="ExternalInput").ap()
    out_pt = nc.dram_tensor("out_pt", [D, S], DTBF, kind="ExternalOutput").ap()

    EXP = mybir.ActivationFunctionType.Exp
    IDENT = mybir.ActivationFunctionType.Identity

    with tile.TileContext(nc) as tc, ExitStack() as ctx:
        consts = ctx.enter_context(tc.tile_pool(name="consts", bufs=1))
        xq_pool = ctx.enter_context(tc.tile_pool(name="xq", bufs=1))
        xk_pool = ctx.enter_context(tc.tile_pool(name="xk", bufs=1))
        xv_pool = ctx.enter_context(tc.tile_pool(name="xv", bufs=2))
        qkv_pool = ctx.enter_context(tc.tile_pool(name="qkv", bufs=1))
        et_pool = ctx.enter_context(tc.tile_pool(name="et", bufs=3))
        rc_pool = ctx.enter_context(tc.tile_pool(name="rc", bufs=2))
        out_pool = ctx.enter_context(tc.tile_pool(name="osb", bufs=6))
        ps_s = ctx.enter_context(tc.tile_pool(name="ps_s", bufs=2, space="PSUM"))
        ps_acc = ctx.enter_context(tc.tile_pool(name="ps_acc", bufs=1, space="PSUM"))
        ps_op = ctx.enter_context(tc.tile_pool(name="ps_op", bufs=2, space="PSUM"))

        # ---- weight / bias SBUF tiles ----
        wq_sb = consts.tile([128, KT, DL], DTBF)
        wk_sb = consts.tile([128, KT, DL], DTBF)
        wv_sb = consts.tile([128, KT, DL], DTBF)
        wo_sb = consts.tile([128, NHP, D], DTBF)
        bq_sb = consts.tile([128, 4], FP32)
        bo_sb = consts.tile([128, 8], FP32)

        # chunk-major free dims: DMA destinations are fully contiguous
        xqt = xq_pool.tile([128, QC, KT, 512], DTBF)
        xkt = xk_pool.tile([128, QC, KT, 512], DTBF)

        # qT/kT are 2-slot rings over head pairs: pass hp reads slot hp%2 while
        # the fills for pass hp+1 write slot (hp+1)%2
        qT_sb = qkv_pool.tile([128, 2, S], DTBF)
        kT_sb = qkv_pool.tile([128, 2, S], DTBF)
        # [ones | V] per head: vp_sb[:, tt, h, 0:64] = 1.0, [.., 64:128] = V.
        # ones first so the softmax denominator lands on PSUM partitions 0:64 —
        # the custom-DVE reciprocal misreads PSUM at partition base 64 on HW.
        vp_sb = qkv_pool.tile([128, TT, 8, 128], DTBF)
        a_sb = qkv_pool.tile([128, NHP, S], DTBF)

        nc.vector.memset(vp_sb[:, :, :, 0:64], 1.0)

        # lower-triangle-inclusive (k <= q) binary mask for diagonal tiles
        tri_sb = consts.tile([128, 2, 128], DTBF)
        nc.gpsimd.memset(tri_sb[:], 1.0)
        for h2 in range(2):
            nc.gpsimd.affine_select(
                out=tri_sb[:, h2, :],
                in_=tri_sb[:, h2, :],
                compare_op=mybir.AluOpType.is_ge,
                fill=0.0,
                base=0,
                pattern=[[1, 128]],
                channel_multiplier=-1,
            )

        # ---- DMA issue order == consumption order --------------------------
        # one contiguous transfer per chunk, spread over the three DGE queues
        # (sync / scalar / gpsimd) so the prelude inputs land in ~15us.
        # xv chunks 2/3 are allocated lazily (after chunk 0/1 readers exist)
        # to keep the ring-reuse ordering acyclic.
        xv_chunks = [
            xv_pool.tile([128, KT, 512], DTBF, tag="xv", name=f"xv{i}") for i in range(2)
        ]
        # ~105GB/s per DGE queue: spread the critical-path chunks so each
        # queue's k-th item is needed no sooner than ~(9 + 10*k)us
        nc.gpsimd.dma_start(wq_sb[:], wq_t[:])
        nc.gpsimd.dma_start(xkt[:, 0], xk[0])
        nc.gpsimd.dma_start(bq_sb[:], bq_t[:])
        nc.gpsimd.dma_start(bo_sb[:], bo_t[:])
        nc.gpsimd.dma_start(xv_chunks[1][:], xv[1])
        nc.gpsimd.dma_start(xkt[:, 2], xk[2])
        nc.sync.dma_start(wk_sb[:], wk_t[:])
        nc.sync.dma_start(xv_chunks[0][:], xv[0])
        nc.sync.dma_start(xkt[:, 1], xk[1])
        nc.sync.dma_start(xqt[:, 2], xq[2])
        nc.sync.dma_start(xkt[:, 3], xk[3])
        nc.sync.dma_start(wo_sb[:], wo_t[:])
        nc.scalar.dma_start(xqt[:, 0], xq[0])
        nc.scalar.dma_start(wv_sb[:], wv_t[:])
        nc.scalar.dma_start(xqt[:, 1], xq[1])
        nc.scalar.dma_start(xqt[:, 3], xq[3])

        # ---- projection unit emitters --------------------------------------
        def qp_unit(ot, tcid, ev_scalar):
            ps = ps_op.tile([128, 512], FP32, tag="op", name="ps")
            for k in range(KT):
                nc.tensor.matmul(
                    ps[:],
                    wq_sb[:, k, ot * 128 : (ot + 1) * 128],
                    xqt[:, tcid, k, :],
                    start=(k == 0),
                    stop=(k == KT - 1),
                )
            dst = qT_sb[:, ot % 2, tcid * 512 : (tcid + 1) * 512]
            if ev_scalar:
                nc.scalar.activation(dst, ps[:], IDENT, bias=bq_sb[:, ot : ot + 1], scale=1.0)
            else:
                nc.vector.tensor_scalar_add(dst, ps[:], bq_sb[:, ot : ot + 1])

        def kp_unit(ot, tcid, ev_scalar):
            ps = ps_op.tile([128, 512], FP32, tag="op", name="ps")
            for k in range(KT):
                nc.tensor.matmul(
                    ps[:],
                    wk_sb[:, k, ot * 128 : (ot + 1) * 128],
                    xkt[:, tcid, k, :],
                    start=(k == 0),
                    stop=(k == KT - 1),
                )
            dst = kT_sb[:, ot % 2, tcid * 512 : (tcid + 1) * 512]
            if ev_scalar:
                nc.scalar.copy(dst, ps[:])
            else:
                nc.vector.tensor_copy(dst, ps[:])

        def vp_unit(tt, ev_scalar):
            xvc = xv_chunks[tt // 4]
            ps = ps_op.tile([128, 512], FP32, tag="op", name="ps")
            for k in range(KT):
                nc.tensor.matmul(
                    ps[:],
                    xvc[:, k, (tt % 4) * 128 : (tt % 4 + 1) * 128],
                    wv_sb[:, k, :],
                    start=(k == 0),
                    stop=(k == KT - 1),
                )
            dst = vp_sb[:, tt, :, 64:128]
            if ev_scalar:
                nc.scalar.copy(dst, ps[:])
            else:
                nc.vector.tensor_copy(dst, ps[:])

        def op_unit(qc, od):
            ps = ps_op.tile([128, 512], FP32, tag="op", name="ps")
            for hp in range(NHP):
                nc.tensor.matmul(
                    ps[:],
                    wo_sb[:, hp, od * 128 : (od + 1) * 128],
                    a_sb[:, hp, qc * 512 : (qc + 1) * 512],
                    start=(hp == 0),
                    stop=(hp == NHP - 1),
                )
            osb = out_pool.tile([128, 512], DTBF, tag="osb", name="osb")
            nc.vector.tensor_scalar_add(osb[:], ps[:], bo_sb[:, od : od + 1])
            eng = nc.sync if od % 2 == 0 else nc.gpsimd
            eng.dma_start(
                out_pt[od * 128 : (od + 1) * 128, qc * 512 : (qc + 1) * 512], osb[:]
            )

        # ---- PE warm-up: a short junk-matmul burst while input DMAs stream,
        # so HAM is at 8/8 when the real pipeline starts ----------------------
        warm = consts.tile([128, 512], DTBF)
        nc.vector.memset(warm[:], 0.0)
        wps = ps_s.tile([128, 2, 512], FP32, tag="ps_s", name="warmps")
        for i in range(16):
            nc.tensor.matmul(wps[:, 0, :], warm[:, 0:128], warm[:], start=True, stop=True)

        # ---- prelude: just enough projection for attn(0,0) ------------------
        qp_unit(0, 0, ev_scalar=True)
        kp_unit(0, 0, ev_scalar=True)
        for tt in range(4):
            vp_unit(tt, ev_scalar=True)
        xv_chunks.append(xv_pool.tile([128, KT, 512], DTBF, tag="xv", name="xv2"))
        nc.scalar.dma_start(xv_chunks[2][:], xv[2])

        # ---- fill pool ------------------------------------------------------
        fills = _Fills()
        QP_C, KP_C, VP_C, OP_C = 2200.0, 2200.0, 2200.0, 1550.0

        def add_pass0():
            f = fills
            f.add("kp0_1", KP_C, lambda: kp_unit(0, 1, False))
            f.add("qp0_1", QP_C, lambda: qp_unit(0, 1, False))
            f.add("vp4", VP_C, lambda: vp_unit(4, False))
            f.add("vp5", VP_C, lambda: vp_unit(5, False))
            f.add("vp6", VP_C, lambda: vp_unit(6, False))
            f.add("vp7", VP_C, lambda: vp_unit(7, False))

            def _xvc3():
                xv_chunks.append(
                    xv_pool.tile([128, KT, 512], DTBF, tag="xv", name="xv3")
                )
                nc.gpsimd.dma_start(xv_chunks[3][:], xv[3])

            f.add("xvc3", 0.0, _xvc3)
            f.add("qp1_0", QP_C, lambda: qp_unit(1, 0, False))
            f.add("kp1_0", KP_C, lambda: kp_unit(1, 0, False))
            f.add("qp0_2", QP_C, lambda: qp_unit(0, 2, False))
            f.add("kp0_2", KP_C, lambda: kp_unit(0, 2, False))
            f.add("vp8", VP_C, lambda: vp_unit(8, False))
            f.add("vp9", VP_C, lambda: vp_unit(9, False))
            f.add("vp10", VP_C, lambda: vp_unit(10, False))
            f.add("vp11", VP_C, lambda: vp_unit(11, False))
            f.add("qp1_1", QP_C, lambda: qp_unit(1, 1, False))
            f.add("kp1_1", KP_C, lambda: kp_unit(1, 1, False))
            f.add("qp0_3", QP_C, lambda: qp_unit(0, 3, False))
            f.add("kp0_3", KP_C, lambda: kp_unit(0, 3, False))
            for tt in range(12, 16):
                f.add(f"vp{tt}", VP_C, lambda tt=tt: vp_unit(tt, False))
            for tcid in range(2, 4):
                f.add(f"qp1_{tcid}", QP_C, lambda t=tcid: qp_unit(1, t, False))
                f.add(f"kp1_{tcid}", KP_C, lambda t=tcid: kp_unit(1, t, False))

        def add_passN(hp_next):
            # emitted as pass (hp_next-1) fills; evac on vector
            for tcid in range(4):
                fills.add(
                    f"qp{hp_next}_{tcid}", QP_C, lambda o=hp_next, t=tcid: qp_unit(o, t, False)
                )
                fills.add(
                    f"kp{hp_next}_{tcid}", KP_C, lambda o=hp_next, t=tcid: kp_unit(o, t, False)
                )

        add_pass0()

        # ---- attention block for one (query chunk, head pair) ---------------
        def attn(qc, hp, credit_scale=1.0):
            jmax = 4 * qc + 3 if causal else TT - 1
            q0 = qc * 512
            pso = ps_acc.tile([128, 2, 512], FP32, tag="acc", name="pso")

            def offof(j):
                r = j - 4 * qc if causal else -1
                return 128 * r if r >= 0 else 0

            def scores(j):
                off = offof(j)
                pss = ps_s.tile([128, 2, 512], FP32, tag="ps_s", name="pss")
                for h2 in range(2):
                    nc.tensor.matmul(
                        pss[:, h2, off:512],
                        kT_sb[h2 * 64 : (h2 + 1) * 64, hp % 2, j * 128 : (j + 1) * 128],
                        qT_sb[h2 * 64 : (h2 + 1) * 64, hp % 2, q0 + off : q0 + 512],
                        start=True,
                        stop=True,
                    )
                et = et_pool.tile([128, 2, 512], DTBF, tag="et", name="et")
                nc.scalar.activation(et[:, :, off:], pss[:, :, off:], EXP, scale=0.125)
                if off or (causal and j == 4 * qc):
                    # zero where k (partition) > q (free col), both heads
                    nc.vector.tensor_mul(
                        et[:, :, off : off + 128], et[:, :, off : off + 128], tri_sb[:]
                    )
                return et

            et_next = scores(0)
            for j in range(jmax + 1):
                off = offof(j)
                et = et_next
                if j < jmax:
                    et_next = scores(j + 1)
                for h2 in range(2):
                    # rows 0:64 accumulate the softmax denominator (ones
                    # block), rows 64:128 attn@V.  Causally-trimmed widths on
                    # interleaved chains; per-element has_written semantics
                    # make this safe but the sim's zero-region tracker
                    # can't express it.
                    nc.tensor.matmul(
                        pso[:, h2, off:512],
                        vp_sb[:, j, 2 * hp + h2, :],
                        et[:, h2, off:],
                        start=(j == 0),
                        stop=(j == jmax),
                        skip_group_check=True,
                    )
                w = 512 - off
                fills.step((0.42 * w + 47.0) * credit_scale)
            # normalize: 1/den on DVE (recip ~51 ULP), then scale the V rows
            rc = rc_pool.tile([128, 2, 512], FP32, tag="rc", name="rc")
            nc.vector.reciprocal_approx_fast(rc[0:64, :, :], pso[0:64, :, :])
            for h2 in range(2):
                nc.vector.tensor_mul(
                    a_sb[h2 * 64 : (h2 + 1) * 64, hp, qc * 512 : (qc + 1) * 512],
                    pso[64:128, h2, :],
                    rc[0:64, h2, :],
                )

        # ---- main pipeline: head-pair passes --------------------------------
        def barrier(qc, hp):
            keys = []
            if causal:
                tt_hi = 4 * qc + 3
                tc_hi = qc
            else:
                tt_hi = TT - 1
                tc_hi = 3
            if hp == 0:
                keys += [f"vp{t}" for t in range(4, tt_hi + 1)]
                keys += [f"kp0_{t}" for t in range(1, tc_hi + 1)]
                if qc >= 1:
                    keys += [f"qp0_{qc}"]
            else:
                keys += [f"qp{hp}_{qc}"]
                keys += [f"kp{hp}_{t}" for t in range(0, tc_hi + 1)]
            return keys

        for hp in range(NHP):
            if 1 <= hp < NHP - 1:
                # pass hp+1's q/k projections (pass 1's are already in the
                # pass-0 list via add_pass0)
                add_passN(hp + 1)
            for qc in range(QC):
                fills.run_until(*barrier(qc, hp))
                attn(qc, hp, credit_scale=(3.0 if hp == NHP - 1 else 1.0))
                if hp == NHP - 1 and qc < QC - 1:
                    # finished chunk's o-proj becomes pass-3 fill work
                    for od in range(8):
                        fills.add(f"op{qc}_{od}", OP_C, lambda q=qc, o=od: op_unit(q, o))

        fills.drain()

        if _DEBUG_DUMP:
            dbg_a = nc.dram_tensor("dbg_a", [128, NHP, S], DTBF, kind="ExternalOutput").ap()
            dbg_q = nc.dram_tensor("dbg_q", [128, 2, S], DTBF, kind="ExternalOutput").ap()
            dbg_k = nc.dram_tensor("dbg_k", [128, 2, S], DTBF, kind="ExternalOutput").ap()
            dbg_v = nc.dram_tensor("dbg_v", [128, TT, 8, 128], DTBF, kind="ExternalOutput").ap()
            nc.sync.dma_start(dbg_a[:], a_sb[:])
            nc.sync.dma_start(dbg_q[:], qT_sb[:])
            nc.sync.dma_start(dbg_k[:], kT_sb[:])
            nc.sync.dma_start(dbg_v[:], vp_sb[:])

        # ---- tail: last chunk's o-proj with all 8 PSUM banks as chains ------
        fin = [ps_s.tile([128, 2, 512], FP32, tag="ps_s", name=f"fin{i}") for i in range(2)]
        fin_acc = ps_acc.tile([128, 2, 512], FP32, tag="acc", name="fin_acc")
        fin_op = [ps_op.tile([128, 512], FP32, tag="op", name=f"finop{i}") for i in range(2)]
        qc = QC - 1
        chains = [
            fin[0][:, 0, :], fin[0][:, 1, :], fin[1][:, 0, :], fin[1][:, 1, :],
            fin_acc[:, 0, :], fin_acc[:, 1, :], fin_op[0][:], fin_op[1][:],
        ]
        for hp in range(NHP):
            for od in range(8):
                nc.tensor.matmul(
                    chains[od],
                    wo_sb[:, hp, od * 128 : (od + 1) * 128],
                    a_sb[:, hp, qc * 512 : (qc + 1) * 512],
                    start=(hp == 0),
                    stop=(hp == NHP - 1),
                )
        for od in range(8):
            osb = out_pool.tile([128, 512], DTBF, tag="osb", name="osb")
            if od % 2 == 0:
                nc.vector.tensor_scalar_add(osb[:], chains[od], bo_sb[:, od : od + 1])
            else:
                nc.scalar.activation(osb[:], chains[od], IDENT, bias=bo_sb[:, od : od + 1], scale=1.0)
            eng = (nc.sync, nc.gpsimd, nc.scalar)[od % 3]
            eng.dma_start(
                out_pt[od * 128 : (od + 1) * 128, qc * 512 : (qc + 1) * 512], osb[:]
            )


_CACHE = {}


def _get_compiled(causal: bool):
    key = bool(causal)
    if key not in _CACHE:
        nc = bacc.Bacc("TRN2", target_bir_lowering=False, debug=False, num_devices=NCORES)
        _emit(nc, causal=key)
        nc.compile()
        _CACHE[key] = nc
    return _CACHE[key]


def make_in_maps(query, key, value, w_q, b_q, w_k, b_k, w_v, b_v, w_o, b_o):
    """Build the per-core input maps (host-side sharding + layout prep)."""
    in_maps = []
    # b_v folds into the output bias: softmax rows sum to 1, so
    # attn(V + b_v) = attn(V) + b_v, and (A + b_v) @ w_o.T = A @ w_o.T + w_o @ b_v.
    # b_k drops entirely: scores shift constant along k cancels in softmax.
    bo_eff = (b_o + w_o.astype(np.float64) @ b_v.astype(np.float64)).astype(np.float32)
    for c in range(NCORES):
        b, hg = divmod(c, 2)
        sl = slice(hg * DL, (hg + 1) * DL)
        bo_core = bo_eff if hg == 0 else np.zeros_like(bo_eff)
        in_maps.append(
            {
                "xq_t": np.ascontiguousarray(
                    query[b].T.reshape(KT, 128, QC, 512).transpose(2, 1, 0, 3)).astype(BF16),
                "xk_t": np.ascontiguousarray(
                    key[b].T.reshape(KT, 128, QC, 512).transpose(2, 1, 0, 3)).astype(BF16),
                "xv_t": np.ascontiguousarray(
                    value[b].T.reshape(KT, 128, QC, 512).transpose(2, 1, 0, 3)).astype(BF16),
                "wq_p": np.ascontiguousarray(
                    w_q[sl, :].T.reshape(KT, 128, DL).transpose(1, 0, 2)).astype(BF16),
                "wk_p": np.ascontiguousarray(
                    w_k[sl, :].T.reshape(KT, 128, DL).transpose(1, 0, 2)).astype(BF16),
                "wv_p": np.ascontiguousarray(
                    w_v[sl, :].T.reshape(KT, 128, DL).transpose(1, 0, 2)).astype(BF16),
                "wo_p": np.ascontiguousarray(
                    w_o[:, sl].T.reshape(NHP, 128, D).transpose(1, 0, 2)).astype(BF16),
                "bq_t": np.ascontiguousarray(b_q[sl].reshape(4, 128).T).astype(np.float32),
                "bo_t": np.ascontiguousarray(bo_core.reshape(8, 128).T).astype(np.float32),
            }
        )
    return in_maps


def _mask_is_causal(mask):
    m = np.asarray(mask).reshape(S, S)
    return bool(np.array_equal(m, np.triu(np.ones((S, S), bool), k=1)))


def _mask_is_empty(mask):
    return not np.asarray(mask).any()


def kernel(query, key, value, mask, w_q, b_q, w_k, b_k, w_v, b_v, w_o, b_o, **_unused):
    query = np.asarray(query, np.float32)
    key = np.asarray(key, np.float32)
    value = np.asarray(value, np.float32)
    if _mask_is_causal(mask):
        causal = True
    elif _mask_is_empty(mask):
        causal = False
    else:
        raise NotImplementedError("only causal or empty masks are supported")

    nc = _get_compiled(causal)
    in_maps = make_in_maps(
        query, key, value,
        np.asarray(w_q, np.float32), np.asarray(b_q, np.float32),
        np.asarray(w_k, np.float32), np.asarray(b_k, np.float32),
        np.asarray(w_v, np.float32), np.asarray(b_v, np.float32),
        np.asarray(w_o, np.float32), np.asarray(b_o, np.float32),
    )
    res = bass_utils.run_bass_kernel_spmd(nc, in_maps, core_ids=list(range(NCORES)))
    out = np.empty((B, S, D), np.float32)
    for b in range(B):
        acc = (
            res.results[2 * b]["out_pt"].astype(np.float32)
            + res.results[2 * b + 1]["out_pt"].astype(np.float32)
        )
        out[b] = acc.T
    return out
